# revision 12
# baseline (speedup 1.0000x reference)
"""Trainium2 Bass kernel: MultiHeadTransformerEncoder layer, 8-core data parallel.

Sharding: core c handles batch b=c//2, sequence half h=c%2 (1024 query tokens).
K/V are computed for the full 2048-token sequence of that batch element
(duplicated across the core pair) so no collectives are needed.

Layout strategy:
  - activations enter feature-major (xT [d, tokens]) so Q/K projections come out
    as qT/kT [feat, tokens] (the layout attention needs) without any transposes
  - V is computed token-major [tokens, feat] (lhsT = xT tile, rhs = wv)
  - scores are computed transposed ([key, query]), softmax denominator comes from
    an extra all-ones column in the AV matmul lhsT (M=65), normalization happens
    as a per-column scale on the ctx PSUM tile
  - all matmuls run in float32r (relaxed fp32, 1 cycle/row, ~1.5e-4 rel err)
"""

import sys
import numpy as np

sys.path.insert(0, '/opt/trn_rl_repo')

B, S, D = 4, 2048, 1024
H, DK, F = 16, 64, 4096
P = 128
SL = 1024            # local (query) tokens per core
NCORES = 8
EPS = 1e-5

_cached_nc = None


def _build(num_devices=NCORES):
    import concourse.bass as bass
    import concourse.bacc as bacc
    import concourse.mybir as mybir
    import concourse.tile as tile
    from concourse.masks import make_identity

    f32 = mybir.dt.float32
    f32r = mybir.dt.float32r
    AF = mybir.ActivationFunctionType
    ALU = mybir.AluOpType

    nc = bacc.Bacc('TRN2', target_bir_lowering=False, debug=False,
                   num_devices=num_devices)

    dt_in = lambda n, shp: nc.dram_tensor(n, shp, f32, kind='ExternalInput').ap()
    xT_d = dt_in('xT', (D, S))
    xtok_d = dt_in('xtok', (SL, D))
    wq_d = dt_in('wq', (D, D))
    wk_d = dt_in('wk', (D, D))
    wv_d = dt_in('wv', (D, D))
    wo_d = dt_in('wo', (D, D))
    w1p_d = dt_in('w1p', (32, P, 1024))     # host-prepped [f_tile, pi, po*128+fi]
    w2_d = dt_in('w2', (F, D))
    bq_d = dt_in('bqp', (P, 8))             # bq[po*128+pi] -> [pi, po]
    b1_d = dt_in('b1p', (P, 32))
    bvr_d = dt_in('bvr', (1, D))
    bor_d = dt_in('bor', (1, D))
    b2r_d = dt_in('b2r', (1, D))
    g1_d = dt_in('g1r', (1, D))
    be1_d = dt_in('be1r', (1, D))
    g2_d = dt_in('g2r', (1, D))
    be2_d = dt_in('be2r', (1, D))
    onesr_d = dt_in('onesr', (1, P))
    onesv_d = dt_in('onesv', (P, 64))
    out_d = nc.dram_tensor('out', (SL, D), f32, kind='ExternalOutput').ap()

    rr = lambda ap: ap.rearrange('(po pi) n -> pi po n', pi=P)

    with tile.TileContext(nc) as tc:
        with tc.tile_pool(name='const', bufs=1) as cst, \
             tc.tile_pool(name='dramp', bufs=1, space='DRAM') as dramp:
            onesr_t = cst.tile([1, P], f32r)
            nc.sync.dma_start(onesr_t[:], onesr_d.bitcast(f32r))
            bq_t = cst.tile([P, 8], f32)
            nc.sync.dma_start(bq_t[:], bq_d)
            b1_t = cst.tile([P, 32], f32)
            nc.sync.dma_start(b1_t[:], b1_d)
            bvr_t = cst.tile([1, D], f32r)
            nc.sync.dma_start(bvr_t[:], bvr_d.bitcast(f32r))
            bor_t = cst.tile([1, D], f32r)
            nc.sync.dma_start(bor_t[:], bor_d.bitcast(f32r))
            b2r_t = cst.tile([1, D], f32r)
            nc.sync.dma_start(b2r_t[:], b2r_d.bitcast(f32r))
            g1_t = cst.tile([P, D], f32)
            nc.sync.dma_start(g1_t[:], g1_d.to_broadcast([P, D]))
            be1_t = cst.tile([P, D], f32)
            nc.sync.dma_start(be1_t[:], be1_d.to_broadcast([P, D]))
            g2_t = cst.tile([P, D], f32)
            nc.sync.dma_start(g2_t[:], g2_d.to_broadcast([P, D]))
            be2_t = cst.tile([P, D], f32)
            nc.sync.dma_start(be2_t[:], be2_d.to_broadcast([P, D]))
            ident_t = cst.tile([P, P], f32)
            make_identity(nc, ident_t)
            eps_t = cst.tile([P, 1], f32)
            nc.vector.memset(eps_t[:], EPS)

            ctx_dram = dramp.tile([H, DK, SL], f32r)
            x1_dram = dramp.tile([8, P, D], f32)

            # ============ attention phase ============
            with tc.tile_pool(name='xtp', bufs=1) as xtp, \
                 tc.tile_pool(name='grp', bufs=2) as grp, \
                 tc.tile_pool(name='wst', bufs=2) as wst, \
                 tc.tile_pool(name='exp', bufs=3) as exq, \
                 tc.tile_pool(name='stg', bufs=2) as stg, \
                 tc.tile_pool(name='ppj', bufs=2, space='PSUM') as ppj, \
                 tc.tile_pool(name='psc', bufs=2, space='PSUM') as psc, \
                 tc.tile_pool(name='pcx', bufs=2, space='PSUM') as pcx:
                xT_t = xtp.tile([P, 8, S], f32r)
                nc.sync.dma_start(xT_t[:], rr(xT_d).bitcast(f32r))

                for g in range(4):
                    f0 = g * 256
                    kT_g = grp.tile([P, 2, S], f32r, name='kT_g', bufs=1)
                    qT_g = grp.tile([P, 2, SL], f32r, name='qT_g', bufs=1)
                    Vaug = grp.tile([P, 16, 4, 65], f32r, name='Vaug')
                    nc.sync.dma_start(
                        Vaug[:, :, :, 0].rearrange('p a b -> p (a b)'),
                        onesv_d.bitcast(f32r))
                    wv_t = wst.tile([P, 8, 256], f32r, name='wv_t', bufs=1)
                    nc.sync.dma_start(wv_t[:],
                                      rr(wv_d)[:, :, f0:f0 + 256].bitcast(f32r))
                    for j in range(2):
                        fc = f0 + j * 128
                        wk_t = wst.tile([P, 8, P], f32r, name='wk_t')
                        nc.sync.dma_start(
                            wk_t[:], rr(wk_d)[:, :, fc:fc + 128].bitcast(f32r))
                        for t4 in range(4):
                            pk = ppj.tile([P, 512], f32, name='pk', tag='pj')
                            for kd in range(8):
                                nc.tensor.matmul(
                                    pk[:], wk_t[:, kd, :],
                                    xT_t[:, kd, t4 * 512:(t4 + 1) * 512],
                                    start=(kd == 0), stop=(kd == 7))
                            nc.vector.tensor_copy(
                                kT_g[:, j, t4 * 512:(t4 + 1) * 512], pk[:])
                        wq_t = wst.tile([P, 8, P], f32r, name='wq_t')
                        nc.sync.dma_start(
                            wq_t[:], rr(wq_d)[:, :, fc:fc + 128].bitcast(f32r))
                        for sc in range(2):
                            pq = ppj.tile([P, 512], f32, name='pq', tag='pj')
                            for kd in range(8):
                                nc.tensor.matmul(
                                    pq[:], wq_t[:, kd, :],
                                    xT_t[:, kd, sc * 512:(sc + 1) * 512],
                                    start=(kd == 0), stop=(kd == 7))
                            nc.vector.tensor_scalar(
                                qT_g[:, j, sc * 512:(sc + 1) * 512], pq[:],
                                bq_t[:, g * 2 + j:g * 2 + j + 1], None, ALU.add)
                    for tt in range(16):
                        pv = ppj.tile([P, 512], f32, name='pv', tag='pj')
                        pvv = pv[:, 0:256]
                        for kd in range(8):
                            nc.tensor.matmul(
                                pvv, xT_t[:, kd, tt * 128:(tt + 1) * 128],
                                wv_t[:, kd, :], start=(kd == 0), stop=False)
                        nc.tensor.matmul(pvv, onesr_t[:],
                                         bvr_t[:, f0:f0 + 256],
                                         start=False, stop=True)
                        nc.vector.tensor_copy(
                            Vaug[:, tt, :, 1:65],
                            pv[:, 0:256].rearrange('p (h d) -> p h d', h=4))
                    # -- attention for the 4 heads of this group --
                    for pr in range(2):
                        for sc in range(2):
                            pca = pcx.tile([P, 512], f32, name='pca', tag='pc')
                            pcb = pcx.tile([P, 512], f32, name='pcb', tag='pc')
                            for tt in range(16):
                                ps_t = psc.tile([P, 1024], f32, name='ps_t')
                                nc.tensor.matmul(
                                    ps_t[:, 0:512],
                                    kT_g[0:64, pr, tt * 128:(tt + 1) * 128],
                                    qT_g[0:64, pr, sc * 512:(sc + 1) * 512],
                                    start=True, stop=True, tile_position=(0, 0))
                                nc.tensor.matmul(
                                    ps_t[:, 512:1024],
                                    kT_g[64:128, pr, tt * 128:(tt + 1) * 128],
                                    qT_g[64:128, pr, sc * 512:(sc + 1) * 512],
                                    start=True, stop=True, tile_position=(64, 0))
                                et = exq.tile([P, 1024], f32r, name='et')
                                nc.scalar.activation(et[:], ps_t[:], AF.Exp,
                                                     scale=0.125)
                                nc.tensor.matmul(
                                    pca[0:65, :], Vaug[:, tt, 2 * pr, :],
                                    et[:, 0:512],
                                    start=(tt == 0), stop=(tt == 15))
                                nc.tensor.matmul(
                                    pcb[0:65, :], Vaug[:, tt, 2 * pr + 1, :],
                                    et[:, 512:1024],
                                    start=(tt == 0), stop=(tt == 15))
                            for hh, pc in ((2 * pr, pca), (2 * pr + 1, pcb)):
                                zr = stg.tile([1, 512], f32, name='zr')
                                nc.vector.reciprocal(zr[0:1, :], pc[0:1, :])
                                zb = stg.tile([P, 512], f32, name='zb')
                                nc.gpsimd.partition_broadcast(zb[0:65, :],
                                                              zr[0:1, :])
                                st_t = stg.tile([65, 512], f32r, name='st_t')
                                nc.vector.tensor_tensor(st_t[0:65, :],
                                                        pc[0:65, :],
                                                        zb[0:65, :], ALU.mult)
                                nc.sync.dma_start(
                                    ctx_dram[g * 4 + hh, :,
                                             sc * 512:(sc + 1) * 512],
                                    st_t[1:65, :])

            # ============ wo projection + LN1 + transpose ============
            with tc.tile_pool(name='x1tp', bufs=1) as x1tp:
                x1T = x1tp.tile([P, 8, SL], f32r)
                with tc.tile_pool(name='mid', bufs=1) as mid, \
                     tc.tile_pool(name='mids', bufs=3) as mids, \
                     tc.tile_pool(name='pmo', bufs=2, space='PSUM') as pmo, \
                     tc.tile_pool(name='pmt', bufs=2, space='PSUM') as pmt:
                    wo_t = mid.tile([P, 8, D], f32r)
                    nc.sync.dma_start(wo_t[:], rr(wo_d).bitcast(f32r))
                    ctxr = mid.tile([P, 8, SL], f32r)
                    for kd in range(8):
                        nc.sync.dma_start(
                            ctxr[:, kd, :],
                            ctx_dram[2 * kd:2 * kd + 2, :, :].rearrange(
                                'h p s -> (h p) s'))
                    for st in range(8):
                        xt_t = mids.tile([P, D], f32, name='xt_t')
                        nc.sync.dma_start(xt_t[:],
                                          xtok_d[st * 128:(st + 1) * 128, :])
                        x1pre = mids.tile([P, D], f32, name='x1pre')
                        for nq in range(2):
                            po = pmo.tile([P, 512], f32, name='po')
                            for kd in range(8):
                                nc.tensor.matmul(
                                    po[:], ctxr[:, kd, st * 128:(st + 1) * 128],
                                    wo_t[:, kd, nq * 512:(nq + 1) * 512],
                                    start=(kd == 0), stop=False)
                            nc.tensor.matmul(po[:], onesr_t[:],
                                             bor_t[:, nq * 512:(nq + 1) * 512],
                                             start=False, stop=True)
                            nc.vector.tensor_tensor(
                                x1pre[:, nq * 512:(nq + 1) * 512], po[:],
                                xt_t[:, nq * 512:(nq + 1) * 512], ALU.add)
                        stats = mids.tile([P, 2, 6], f32, name='stats')
                        nc.vector.bn_stats(stats[:, 0, :], x1pre[:, 0:512])
                        nc.vector.bn_stats(stats[:, 1, :], x1pre[:, 512:1024])
                        mv = mids.tile([P, 2], f32, name='mv')
                        nc.vector.bn_aggr(mv[:], stats[:])
                        sd = mids.tile([P, 1], f32, name='sd')
                        nc.scalar.activation(sd[:], mv[:, 1:2], AF.Sqrt,
                                             bias=eps_t[:])
                        rsd = mids.tile([P, 1], f32, name='rsd')
                        nc.vector.reciprocal(rsd[:], sd[:])
                        nmr = mids.tile([P, 1], f32, name='nmr')
                        nc.vector.scalar_tensor_tensor(nmr[:], mv[:, 0:1], -1.0,
                                                       rsd[:], ALU.mult,
                                                       ALU.mult)
                        x1 = mids.tile([P, D], f32, name='x1')
                        nc.scalar.activation(x1[:], x1pre[:], AF.Identity,
                                             bias=nmr[:], scale=rsd[:])
                        nc.vector.scalar_tensor_tensor(x1[:], x1[:], 1.0,
                                                       g1_t[:], ALU.mult,
                                                       ALU.mult)
                        nc.vector.tensor_tensor(x1[:], x1[:], be1_t[:], ALU.add)
                        nc.sync.dma_start(x1_dram[st], x1[:])
                        for db in range(8):
                            ptp = pmt.tile([P, P], f32, name='ptp')
                            nc.tensor.transpose(
                                ptp[:], x1[:, db * 128:(db + 1) * 128],
                                ident_t[:])
                            nc.vector.tensor_copy(
                                x1T[:, db, st * 128:(st + 1) * 128], ptp[:])

                # ============ FFN + LN2 ============
                with tc.tile_pool(name='ffn', bufs=1) as ffn, \
                     tc.tile_pool(name='htp', bufs=16) as htp, \
                     tc.tile_pool(name='w1s', bufs=2) as w1s, \
                     tc.tile_pool(name='w2s', bufs=2) as w2s, \
                     tc.tile_pool(name='ffs', bufs=2) as ffs:
                    acc = ffn.tile([P, 8, D], f32)
                    for fh in range(2):
                        hts = []
                        with tc.tile_pool(name='pf1', bufs=4,
                                          space='PSUM') as pf1:
                            for ft in range(16):
                                f = fh * 16 + ft
                                w1_t = w1s.tile([P, 1024], f32r, name='w1_t')
                                nc.sync.dma_start(w1_t[:],
                                                  w1p_d[f].bitcast(f32r))
                                hT = htp.tile([P, SL], f32r, name='hT')
                                hts.append(hT)
                                for sc in range(2):
                                    ph = pf1.tile([P, 512], f32, name='ph')
                                    for kd in range(8):
                                        nc.tensor.matmul(
                                            ph[:],
                                            w1_t[:, kd * 128:(kd + 1) * 128],
                                            x1T[:, kd,
                                                sc * 512:(sc + 1) * 512],
                                            start=(kd == 0), stop=(kd == 7))
                                    nc.vector.tensor_scalar(
                                        hT[:, sc * 512:(sc + 1) * 512], ph[:],
                                        b1_t[:, f:f + 1], 0.0, ALU.add, ALU.max)
                        with tc.tile_pool(name='pf2', bufs=8,
                                          space='PSUM') as pf2:
                            for sh in range(2):
                                pfs = {}
                                x1rs = {}
                                for s4 in range(4):
                                    st = sh * 4 + s4
                                    if fh == 0:
                                        x1r = ffs.tile([P, D], f32, name='x1r')
                                        nc.sync.dma_start(x1r[:], x1_dram[st])
                                        x1rs[st] = x1r
                                    for nq in range(2):
                                        pfs[(st, nq)] = pf2.tile([P, 512], f32,
                                                                 name='pf_t')
                                for ft in range(16):
                                    f = fh * 16 + ft
                                    w2_t = w2s.tile([P, D], f32r, name='w2_t')
                                    nc.sync.dma_start(
                                        w2_t[:],
                                        w2_d[f * 128:(f + 1) * 128, :].bitcast(
                                            f32r))
                                    for s4 in range(4):
                                        st = sh * 4 + s4
                                        for nq in range(2):
                                            nc.tensor.matmul(
                                                pfs[(st, nq)][:],
                                                hts[ft][:,
                                                        st * 128:(st + 1) * 128],
                                                w2_t[:,
                                                     nq * 512:(nq + 1) * 512],
                                                start=(ft == 0),
                                                stop=(ft == 15 and fh == 0))
                                for s4 in range(4):
                                    st = sh * 4 + s4
                                    if fh == 0:
                                        for nq in range(2):
                                            nc.vector.tensor_tensor(
                                                acc[:, st,
                                                    nq * 512:(nq + 1) * 512],
                                                pfs[(st, nq)][:],
                                                x1rs[st][:,
                                                         nq * 512:(nq + 1) * 512],
                                                ALU.add)
                                    else:
                                        x2 = ffs.tile([P, D], f32, name='x2')
                                        for nq in range(2):
                                            nc.tensor.matmul(
                                                pfs[(st, nq)][:], onesr_t[:],
                                                b2r_t[:,
                                                      nq * 512:(nq + 1) * 512],
                                                start=False, stop=True)
                                            nc.vector.tensor_tensor(
                                                x2[:, nq * 512:(nq + 1) * 512],
                                                pfs[(st, nq)][:],
                                                acc[:, st,
                                                    nq * 512:(nq + 1) * 512],
                                                ALU.add)
                                        stats = ffs.tile([P, 2, 6], f32,
                                                         name='stats2')
                                        nc.vector.bn_stats(stats[:, 0, :],
                                                           x2[:, 0:512])
                                        nc.vector.bn_stats(stats[:, 1, :],
                                                           x2[:, 512:1024])
                                        mv = ffs.tile([P, 2], f32, name='mv2')
                                        nc.vector.bn_aggr(mv[:], stats[:])
                                        sd = ffs.tile([P, 1], f32, name='sd2')
                                        nc.scalar.activation(sd[:], mv[:, 1:2],
                                                             AF.Sqrt,
                                                             bias=eps_t[:])
                                        rsd = ffs.tile([P, 1], f32, name='rsd2')
                                        nc.vector.reciprocal(rsd[:], sd[:])
                                        nmr = ffs.tile([P, 1], f32, name='nmr2')
                                        nc.vector.scalar_tensor_tensor(
                                            nmr[:], mv[:, 0:1], -1.0, rsd[:],
                                            ALU.mult, ALU.mult)
                                        ot = ffs.tile([P, D], f32, name='ot')
                                        nc.scalar.activation(ot[:], x2[:],
                                                             AF.Identity,
                                                             bias=nmr[:],
                                                             scale=rsd[:])
                                        nc.vector.scalar_tensor_tensor(
                                            ot[:], ot[:], 1.0, g2_t[:],
                                            ALU.mult, ALU.mult)
                                        nc.vector.tensor_tensor(
                                            ot[:], ot[:], be2_t[:], ALU.add)
                                        nc.sync.dma_start(
                                            out_d[st * 128:(st + 1) * 128, :],
                                            ot[:])

    nc.compile()
    return nc


def _build_single():
    return _build(1)


def _prep_in_maps(inputs):
    x = np.ascontiguousarray(inputs['x'], dtype=np.float32)
    f32 = np.float32
    wq = np.ascontiguousarray(inputs['wq'], f32)
    wk = np.ascontiguousarray(inputs['wk'], f32)
    wv = np.ascontiguousarray(inputs['wv'], f32)
    wo = np.ascontiguousarray(inputs['wo'], f32)
    w1 = np.ascontiguousarray(inputs['w1'], f32)
    w2 = np.ascontiguousarray(inputs['w2'], f32)
    # w1p[f_tile, pi, po*128 + fi] = w1[po*128 + pi, f_tile*128 + fi]
    w1p = np.ascontiguousarray(
        w1.reshape(8, 128, 32, 128).transpose(2, 1, 0, 3).reshape(32, 128, 1024))
    shared = {
        'wq': wq, 'wk': wk, 'wv': wv, 'wo': wo, 'w1p': w1p, 'w2': w2,
        'bqp': np.ascontiguousarray(inputs['bq'].reshape(8, 128).T, f32),
        'b1p': np.ascontiguousarray(inputs['b1'].reshape(32, 128).T, f32),
        'bvr': np.ascontiguousarray(inputs['bv'].reshape(1, D), f32),
        'bor': np.ascontiguousarray(inputs['bo'].reshape(1, D), f32),
        'b2r': np.ascontiguousarray(inputs['b2'].reshape(1, D), f32),
        'g1r': np.ascontiguousarray(inputs['g1'].reshape(1, D), f32),
        'be1r': np.ascontiguousarray(inputs['be1'].reshape(1, D), f32),
        'g2r': np.ascontiguousarray(inputs['g2'].reshape(1, D), f32),
        'be2r': np.ascontiguousarray(inputs['be2'].reshape(1, D), f32),
        'onesr': np.ones((1, P), f32),
        'onesv': np.ones((P, 64), f32),
    }
    in_maps = []
    for c in range(NCORES):
        b, h = c // 2, c % 2
        own = x[b, h * SL:(h + 1) * SL, :]
        other = x[b, (1 - h) * SL:(2 - h) * SL, :]
        xT = np.ascontiguousarray(np.concatenate([own, other], axis=0).T)
        m = dict(shared)
        m['xT'] = xT
        m['xtok'] = np.ascontiguousarray(own)
        in_maps.append(m)
    return in_maps


def kernel(**inputs):
    global _cached_nc
    from concourse.bass_utils import run_bass_kernel_spmd
    if _cached_nc is None:
        _cached_nc = _build()
    in_maps = _prep_in_maps(inputs)
    res = run_bass_kernel_spmd(_cached_nc, in_maps,
                               core_ids=list(range(NCORES)))
    out = np.empty((B, S, D), dtype=np.float32)
    for c in range(NCORES):
        b, h = c // 2, c % 2
        out[b, h * SL:(h + 1) * SL, :] = res.results[c]['out']
    return out


def run_traced(inputs, **kw):
    """test.py hook: returns (output, BassKernelResults) with trace enabled."""
    global _cached_nc
    from concourse.bass_utils import run_bass_kernel_spmd
    if _cached_nc is None:
        _cached_nc = _build()
    in_maps = _prep_in_maps(inputs)
    res = run_bass_kernel_spmd(_cached_nc, in_maps,
                               core_ids=list(range(NCORES)), **kw)
    out = np.empty((B, S, D), dtype=np.float32)
    for c in range(NCORES):
        b, h = c // 2, c % 2
        out[b, h * SL:(h + 1) * SL, :] = res.results[c]['out']
    return out, res
